# revision 24
# baseline (speedup 1.0000x reference)
"""Multi-head attention (B=2,S=2048,E=1024,H=16,DK=DV=64) on 8 Trainium2 cores.

Sharding: core c handles batch c//4 and head-group c%4 (4 heads = 2 pairs).
Fully software-pipelined single-pass kernel, engine-balanced around the
ScalarE exp stream (the hard floor: S^2*HL exps per core):

 - Projections in fp8e4 with DoubleRow matmuls (2x PE rate; weights scaled
   x64 into fp8 range, descaled at PSUM evacuation on VectorE, which also
   adds the bias via tensor_scalar's per-partition operand).
 - Scores bf16, two head-chains on disjoint PE row halves (concurrent K=64
   matmuls), 2 k-tiles per step -> one [128,2048] PSUM span, one exp call.
 - Mask applied multiplicatively in-place on VectorE (bf16 2x rate).
 - Ctx bf16 M=65 with a ones column producing the softmax denominator for
   free; both chains accumulate in one [65,1024] PSUM tile.
 - Normalization: reciprocal on VectorE, partition-broadcast on GpSimd,
   fused evac*recip on VectorE; chain B shifted to partitions 64:127 via
   SBUF->SBUF DMA (DVE is lane-locked).
 - Output projection (bf16, K=128) and pair-1 projections interleaved into
   the attention groups so the PE never idles; y returned as bf16 partials,
   host adds partials + bo + bv@Wo (exact: softmax rows sum to 1).
"""

import numpy as np
import ml_dtypes

import concourse.bacc as bacc
import concourse.mybir as mybir
import concourse.tile as tile
from concourse import bass_utils

BF = ml_dtypes.bfloat16
F8 = ml_dtypes.float8_e4m3fn
dt = mybir.dt
Exp = mybir.ActivationFunctionType.Exp
Copy = mybir.ActivationFunctionType.Copy
DR = mybir.MatmulPerfMode.DoubleRow
MUL = mybir.AluOpType.mult
ADD = mybir.AluOpType.add

NCORES = 8
W8SCALE = 64.0       # fp8 weight pre-scale (power of two)


def _emit(nc, tc, inp, y_d, S, E, HL, DK, dbg=None):
    NP = HL // 2              # head pairs (2)
    NT = S // 128             # seq tiles (16)
    EC = E // 128             # contraction chunks (8)
    NQ = S // 512             # q blocks (4)
    QB = 512                  # q block width

    persist = tc.alloc_tile_pool(name="persist", bufs=1)
    qT = [persist.tile([128, S], dt.bfloat16, name=f"qT{p}") for p in range(NP)]
    kT = [persist.tile([128, S], dt.bfloat16, name=f"kT{p}") for p in range(NP)]
    cT = [persist.tile([128, S], dt.bfloat16, name=f"cT{p}") for p in range(NP)]
    # v in natural layout: [seq-part, (t, head, 65)]; col 64 of each head
    # block preset to 1.0 (denominator ones column)
    vA = persist.tile([128, NT, 2 * NP, 65], dt.bfloat16, name="vA")
    nc.gpsimd.memset(vA[:], 1.0)
    neg3 = persist.tile([128, 1], dt.float32, name="neg3")
    nc.gpsimd.memset(neg3[:], -3.0)

    w_sb = {}
    for nm in ("wq", "wk", "wv"):
        w_sb[nm] = [persist.tile([128, 2 * NP * 64], dt.bfloat16, name=f"{nm}{c}")
                    for c in range(EC)]
    b_sb = {}
    for nm in ("bq", "bk"):
        b_sb[nm] = persist.tile([128, NP, 1], dt.float32, name=f"{nm}s")
        for p in range(NP):
            nc.sync.dma_start(b_sb[nm][:, p, :], inp[nm][p])
    wo_sb = [persist.tile([128, E], dt.bfloat16, name=f"wo{p}") for p in range(NP)]
    for p in range(NP):
        nc.sync.dma_start(wo_sb[p][:], inp["wo"][p])
    for nm in ("wk", "wq", "wv"):
        for c in range(EC):
            nc.sync.dma_start(w_sb[nm][c][:], inp[nm][c])

    xs = {}
    for nm in ("xq", "xk", "xv"):
        xs[nm] = [persist.tile([128, S], dt.bfloat16, name=f"{nm}{c}")
                  for c in range(EC)]

    mpool = tc.alloc_tile_pool(name="mask", bufs=4)
    aux = tc.alloc_tile_pool(name="aux", bufs=2, space="PSUM")
    espool = tc.alloc_tile_pool(name="es", bufs=6)
    stpool = tc.alloc_tile_pool(name="st", bufs=2, space="PSUM")
    ctxpool = tc.alloc_tile_pool(name="ctx", bufs=1, space="PSUM")
    npool = tc.alloc_tile_pool(name="nrm", bufs=1)
    ypool = tc.alloc_tile_pool(name="ysb", bufs=2)

    # ---- building-block emitters -------------------------------------------
    def qk_proj_unit(nm, pair, n0):
        """One n0-chunk of a q/k projection for one pair (4 DR MMs + evac)."""
        ps = aux.tile([128, 512], dt.float32, tag="aux", name=f"{nm}p{pair}_{n0}")
        w = w_sb["wq" if nm == "q" else "wk"]
        x = xs["xq" if nm == "q" else "xk"]
        for c in range(EC):
            nc.tensor.matmul(ps[:], w[c][:, 128 * pair:128 * (pair + 1)],
                             x[c][:, n0:n0 + 512],
                             start=(c == 0), stop=(c == EC - 1))
        dst = (qT if nm == "q" else kT)[pair][:, n0:n0 + 512]
        sc = 0.125 if nm == "q" else 1.0
        bias = b_sb["bq" if nm == "q" else "bk"][:, pair, :]
        nc.vector.tensor_scalar(dst, ps[:], sc, bias, MUL, ADD)

    def v_proj_unit(t):
        """v projection for one seq tile, all 4 heads (4 DR MMs + evac)."""
        ps = aux.tile([128, 512], dt.float32, tag="aux", name=f"vp{t}")
        for c in range(EC):
            nc.tensor.matmul(ps[:, 0:256],
                             xs["xv"][c][:, 128 * t:128 * (t + 1)],
                             w_sb["wv"][c][:],
                             start=(c == 0), stop=(c == EC - 1))
        nc.vector.tensor_copy(
            vA[:, t, :, 0:64],
            ps[:, 0:256].rearrange("p (h c) -> p h c", h=2 * NP))

    def outproj_unit(s, n0):
        """One (s-tile, E-chunk) of the output projection (2 MMs + evac + dma)."""
        ps = aux.tile([128, 512], dt.float32, tag="aux", name=f"y{s}_{n0}")
        for p in range(NP):
            nc.tensor.matmul(ps[:], cT[p][:, 128 * s:128 * (s + 1)],
                             wo_sb[p][:, n0:n0 + 512],
                             start=(p == 0), stop=(p == NP - 1))
        ysb = ypool.tile([128, 512], dt.bfloat16, tag="y", name=f"ysb{s}_{n0}")
        if (s + n0 // 512) % 2 == 0:
            nc.vector.tensor_copy(ysb[:], ps[:])
        else:
            nc.scalar.activation(ysb[:], ps[:], Copy)
        nc.sync.dma_start(y_d[128 * s:128 * (s + 1), n0:n0 + 512], ysb[:])

    # mask streamed as half-blocks [128, 8, 512] (4-deep ring, 32KB): the
    # ring refills mid-block so block boundaries see no mask-DMA hole.
    # The whole mask is re-read once per pair (does not fit SBUF at bf16).
    blocks = [(p, qb) for p in range(NP) for qb in range(NQ)]
    halves = [(p, qb, hh) for (p, qb) in blocks for hh in (0, 1)]
    mtiles = {}
    HNT = NT // 2

    def mask_half(idx):
        pair, qb, hh = halves[idx]
        mt = mpool.tile([128, HNT, 512], dt.bfloat16, tag="mask",
                        name=f"mt{pair}_{qb}_{hh}")
        for t in range(HNT):
            nc.sync.dma_start(mt[:, t, :],
                              inp["mask"][:, HNT * hh + t,
                                          512 * qb:512 * qb + 512])
        mtiles[idx] = mt

    # startup DMA interleave, ordered by first-consumer time: each entry is
    # a ~1MB unit (all 8 contraction chunks of one 512-col slice).
    def x_slice(nm, n0):
        for c in range(EC):
            nc.sync.dma_start(xs[nm][c][:, n0:n0 + 512],
                              inp[nm][c][:, n0:n0 + 512])

    x_slice("xk", 0)
    x_slice("xk", 512)
    x_slice("xq", 0)
    mask_half(0)
    x_slice("xq", 512)
    x_slice("xk", 1024)
    x_slice("xv", 0)
    mask_half(1)
    x_slice("xk", 1536)
    x_slice("xq", 1024)
    x_slice("xv", 512)
    x_slice("xq", 1536)
    x_slice("xv", 1024)
    x_slice("xv", 1536)
    mask_half(2)
    mask_req = [3]

    # ---- prologue: pair-0 q/k projections + first v tiles -------------------
    for n0 in range(0, S, 512):
        qk_proj_unit("k", 0, n0)
    for n0 in range(0, S, 512):
        qk_proj_unit("q", 0, n0)
    for t in range(4):
        v_proj_unit(t)

    # infill schedules per (pair, q-block): units emitted between attention
    # groups.  Ordering constraints: v(t) must precede ctx use (block 0 pops
    # 2/group, staying ahead of consumption); outproj for q-block b only
    # after pair-1 norm of block b (so it is scheduled during block b+1).
    infill = {
        (0, 0): [lambda t=t: v_proj_unit(t) for t in range(4, NT)],
        (0, 1): ([lambda n0=n0: qk_proj_unit("k", 1, n0)
                  for n0 in range(0, S, 512)]
                 + [lambda n0=n0: qk_proj_unit("q", 1, n0)
                    for n0 in range(0, S, 512)]),
        (1, 1): [lambda s=s, n0=n0: outproj_unit(s, n0)
                 for s in range(0, 4) for n0 in (0, 512)],
        (1, 2): [lambda s=s, n0=n0: outproj_unit(s, n0)
                 for s in range(4, 8) for n0 in (0, 512)],
        (1, 3): [lambda s=s, n0=n0: outproj_unit(s, n0)
                 for s in range(8, 12) for n0 in (0, 512)],
    }
    tail = [lambda s=s, n0=n0: outproj_unit(s, n0)
            for s in range(NT - 4, NT) for n0 in (0, 512)]

    # ---- main attention loop ------------------------------------------------
    for pair in range(NP):
        for qb in range(NQ):
            q0 = qb * QB
            units = infill.get((pair, qb), [])
            nu = len(units)
            bi = blocks.index((pair, qb))
            ctx2 = ctxpool.tile([65, 1024], dt.float32, tag="ctx",
                                name=f"ctx{pair}_{qb}")
            popped = 0
            for t in range(NT):
                # front-loaded infill: units for resource r must be emitted
                # strictly before their consumer (v(t) before ctx(t))
                target = -(-(nu * (t + 1)) // NT)
                while popped < target and units:
                    units.pop(0)()
                    popped += 1
                # keep the mask half-tile ring 3 ahead of the consumer
                hidx = 2 * bi + t // HNT
                want = hidx + 3
                while mask_req[0] <= want and mask_req[0] < len(halves):
                    mask_half(mask_req[0])
                    mask_req[0] += 1
                mt = mtiles[hidx]
                st = stpool.tile([128, 1024], dt.float32, tag="st",
                                 name=f"st{pair}_{qb}_{t}")
                # scores: chains on disjoint row halves issue concurrently
                for ch in range(2):
                    sub = 64 * ch
                    nc.tensor.matmul(
                        st[:, 512 * ch:512 * ch + 512],
                        kT[pair][sub:sub + 64, 128 * t:128 * (t + 1)],
                        qT[pair][sub:sub + 64, q0:q0 + QB],
                        start=True, stop=True)
                es = espool.tile([128, 1024], dt.bfloat16, tag="es",
                                 name=f"es{pair}_{qb}_{t}")
                nc.scalar.activation(es[:], st[:], Exp, bias=neg3[:])
                esv = es[:].rearrange("p (c n) -> p c n", c=2)
                for ch in range(2):
                    nc.vector.tensor_mul(esv[:, ch], esv[:, ch],
                                         mt[:, t % HNT, :])
                for ch in range(2):
                    h = 2 * pair + ch
                    nc.tensor.matmul(
                        ctx2[:, 512 * ch:512 * ch + 512],
                        vA[:, t, h, 0:65],
                        esv[:, ch], start=(t == 0), stop=(t == NT - 1))
                if t % HNT == HNT - 1:
                    mtiles.pop(hidx)
            # ---- normalization of this q block ------------------------------
            den = npool.tile([1, 1024], dt.float32, tag="dn", name=f"dn{pair}_{qb}")
            nc.vector.tensor_copy(den[:], ctx2[64:65, :])
            recip = npool.tile([1, 1024], dt.float32, tag="rc", name=f"rc{pair}_{qb}")
            nc.vector.reciprocal_approx_fast(recip[:], den[:])
            if dbg is not None and (pair, qb) == (0, 0):
                nc.sync.dma_start(dbg["den0"][:], den[:])
            bcast = npool.tile([64, 1024], dt.float32, tag="bc", name=f"bc{pair}_{qb}")
            nc.gpsimd.partition_broadcast(bcast[:], recip[:])
            nc.vector.tensor_mul(cT[pair][0:64, q0:q0 + QB],
                                 ctx2[0:64, 0:512], bcast[0:64, 0:512])
            tmpb = npool.tile([64, 512], dt.bfloat16, tag="tb", name=f"tb{pair}_{qb}")
            nc.vector.tensor_mul(tmpb[:], ctx2[0:64, 512:1024],
                                 bcast[0:64, 512:1024])
            nc.sync.dma_start(cT[pair][64:128, q0:q0 + QB], tmpb[:])

    for u in tail:
        u()
    if dbg is not None:
        for p in range(NP):
            nc.sync.dma_start(dbg[f"qT{p}"][:], qT[p][:])
            nc.sync.dma_start(dbg[f"kT{p}"][:], kT[p][:])
            nc.sync.dma_start(dbg[f"cT{p}"][:], cT[p][:])
        nc.sync.dma_start(dbg["vA"][:], vA[:].rearrange("p t h c -> p (t h c)"))

    ypool.release()
    npool.release()
    ctxpool.release()
    stpool.release()
    espool.release()
    aux.release()
    mpool.release()
    persist.release()


def _build(S, E, HL, DK):
    NP = HL // 2
    EC = E // 128
    NT = S // 128
    nc = bacc.Bacc("TRN2", target_bir_lowering=False, debug=False,
                   num_devices=NCORES)
    inp = {}
    for nm in ("xq", "xk", "xv"):
        inp[nm] = nc.dram_tensor(nm, [EC, 128, S], dt.bfloat16,
                                 kind="ExternalInput").ap()
    for nm in ("wq", "wk", "wv"):
        inp[nm] = nc.dram_tensor(nm, [EC, 128, 2 * NP * DK], dt.bfloat16,
                                 kind="ExternalInput").ap()
    for nm in ("bq", "bk"):
        inp[nm] = nc.dram_tensor(nm, [NP, 128, 1], dt.float32,
                                 kind="ExternalInput").ap()
    inp["wo"] = nc.dram_tensor("wo", [NP, 128, E], dt.bfloat16,
                               kind="ExternalInput").ap()
    inp["mask"] = nc.dram_tensor("mask", [128, NT, S], dt.bfloat16,
                                 kind="ExternalInput").ap()
    y_d = nc.dram_tensor("y", [S, E], dt.bfloat16, kind="ExternalOutput").ap()

    import os
    dbg = None
    if os.environ.get("K_DBG"):
        dbg = {}
        for p in range(NP):
            for nm in ("qT", "kT", "cT"):
                dbg[f"{nm}{p}"] = nc.dram_tensor(
                    f"dbg_{nm}{p}", [128, S], dt.bfloat16,
                    kind="ExternalOutput").ap()
        dbg["vA"] = nc.dram_tensor("dbg_vA", [128, NT * 2 * NP * 65],
                                   dt.bfloat16, kind="ExternalOutput").ap()
        dbg["den0"] = nc.dram_tensor("dbg_den0", [1, 1024], dt.float32,
                                     kind="ExternalOutput").ap()
    with tile.TileContext(nc) as tc:
        _emit(nc, tc, inp, y_d, S, E, HL, DK, dbg=dbg)
    nc.compile()
    return nc


_CACHE = {}
_TRACE = False
_TRACE_CORES = (0,)
_LAST_RESULT = None


def _get_nc(S, E, HL, DK):
    key = (S, E, HL, DK)
    if key not in _CACHE:
        _CACHE[key] = _build(S, E, HL, DK)
    return _CACHE[key]


_RUNNER_CACHE = {}


def _get_runner(nc):
    """Cached jitted shard_map executable (see bass2jax.run_bass_via_pjrt)."""
    if id(nc) in _RUNNER_CACHE:
        return _RUNNER_CACHE[id(nc)]
    import jax
    import concourse.mybir as _mybir
    from concourse import bass2jax
    from jax.sharding import Mesh, PartitionSpec
    from jax.experimental.shard_map import shard_map

    bass2jax.install_neuronx_cc_hook()
    pid_name = nc.partition_id_tensor.name if nc.partition_id_tensor else None
    in_names, out_names, out_avals, zero_shapes = [], [], [], []
    for alloc in nc.m.functions[0].allocations:
        if not isinstance(alloc, _mybir.MemoryLocationSet):
            continue
        name = alloc.memorylocations[0].name
        if alloc.kind == "ExternalInput":
            if name != pid_name:
                in_names.append(name)
        elif alloc.kind == "ExternalOutput":
            out_names.append(name)
            shape = tuple(alloc.tensor_shape)
            dtype = _mybir.dt.np(alloc.dtype)
            out_avals.append(jax.core.ShapedArray(shape, dtype))
            zero_shapes.append((shape, dtype))
    n_params = len(in_names)
    n_outs = len(out_avals)
    all_names = in_names + out_names
    if pid_name is not None:
        all_names = all_names + [pid_name]

    def _body(*args):
        operands = list(args)
        if pid_name is not None:
            operands.append(bass2jax.partition_id_tensor())
        return tuple(bass2jax._bass_exec_p.bind(
            *operands,
            out_avals=tuple(out_avals),
            in_names=tuple(all_names),
            out_names=tuple(out_names),
            lowering_input_output_aliases=(),
            sim_require_finite=True,
            sim_require_nnan=True,
            nc=nc,
        ))

    devices = jax.devices()[:NCORES]
    mesh = Mesh(np.asarray(devices), ("core",))
    donate = tuple(range(n_params, n_params + n_outs))
    sharded = jax.jit(
        shard_map(_body, mesh=mesh,
                  in_specs=(PartitionSpec("core"),) * (n_params + n_outs),
                  out_specs=(PartitionSpec("core"),) * n_outs,
                  check_rep=False),
        donate_argnums=donate, keep_unused=True)

    def run(in_maps):
        concat_in = [np.concatenate([np.asarray(m[nm]) for m in in_maps], axis=0)
                     for nm in in_names]
        concat_zeros = [np.zeros((NCORES * s[0], *s[1:]), d)
                        for s, d in zero_shapes]
        outs = sharded(*concat_in, *concat_zeros)
        return [
            {nm: np.asarray(outs[i]).reshape(NCORES, *out_avals[i].shape)[c]
             for i, nm in enumerate(out_names)}
            for c in range(NCORES)
        ]

    _RUNNER_CACHE[id(nc)] = run
    return run


def run_sharded(query, key, value, mask, Wq, bq, Wk, bk, Wv, bv, Wo, bo):
    global _LAST_RESULT
    query, key, value = (np.asarray(a, np.float32) for a in (query, key, value))
    mask = np.asarray(mask)
    Wq, bq, Wk, bk, Wv, bv, Wo, bo = (
        np.asarray(a, np.float32) for a in (Wq, bq, Wk, bk, Wv, bv, Wo, bo))

    B, S, E = query.shape
    HDK = Wq.shape[1]
    DKv = 64
    H = HDK // DKv
    GPB = NCORES // B                 # cores per batch (4)
    HL = H // GPB                     # heads per core (4)
    DKL = HL * DKv                    # local head dims (256)
    NP = HL // 2
    CP = E // 256
    NT = S // 128

    nc = _get_nc(S, E, HL, DKv)

    EC = E // 128
    xb = {}
    for b in range(B):
        xb[b] = {
            "xq": np.ascontiguousarray(query[b].T).astype(BF).reshape(EC, 128, S),
            "xk": np.ascontiguousarray(key[b].T).astype(BF).reshape(EC, 128, S),
            "xv": np.ascontiguousarray(value[b].T).astype(BF).reshape(EC, 128, S),
            "mask": np.ascontiguousarray(
                mask[b].T.reshape(NT, 128, S).transpose(1, 0, 2)).astype(BF),
        }

    in_maps = []
    for c in range(NCORES):
        b, g = c // GPB, c % GPB
        sl = slice(g * DKL, (g + 1) * DKL)
        in_maps.append({
            **xb[b],
            "wq": np.ascontiguousarray(Wq[:, sl]).astype(BF).reshape(EC, 128, DKL),
            "wk": np.ascontiguousarray(Wk[:, sl]).astype(BF).reshape(EC, 128, DKL),
            "wv": np.ascontiguousarray(Wv[:, sl]).astype(BF).reshape(EC, 128, DKL),
            "bq": (bq[sl] * 0.125).astype(np.float32).reshape(NP, 128, 1),
            "bk": bk[sl].astype(np.float32).reshape(NP, 128, 1),
            "wo": np.ascontiguousarray(Wo[sl, :]).astype(BF).reshape(NP, 128, E),
        })

    if _TRACE:
        res = bass_utils.run_bass_kernel_spmd(
            nc, in_maps, core_ids=list(range(NCORES)),
            trace=True, trace_cores=list(_TRACE_CORES))
        _LAST_RESULT = res
        results = res.results
    else:
        results = _get_runner(nc)(in_maps)

    y = np.zeros((B, S, E), np.float32)
    for c in range(NCORES):
        y[c // GPB] += np.asarray(results[c]["y"], np.float32)
    y += bo.astype(np.float32) + bv.astype(np.float32) @ Wo
    return y


def kernel(**inputs):
    return run_sharded(
        inputs["query"], inputs["key"], inputs["value"], inputs["mask"],
        inputs["Wq"], inputs["bq"], inputs["Wk"], inputs["bk"],
        inputs["Wv"], inputs["bv"], inputs["Wo"], inputs["bo"])


# revision 25
# speedup vs baseline: 1.2138x; 1.2138x over previous
"""Multi-head attention (B=2,S=2048,E=1024,H=16,DK=DV=64) on 8 Trainium2 cores.

Sharding: core c handles batch c//4 and head-group c%4 (4 heads = 2 pairs).
Fully software-pipelined single-pass kernel, engine-balanced around the
ScalarE exp stream (the hard floor: S^2*HL exps per core):

 - Projections in fp8e4 with DoubleRow matmuls (2x PE rate; weights scaled
   x64 into fp8 range, descaled at PSUM evacuation on VectorE, which also
   adds the bias via tensor_scalar's per-partition operand).
 - Scores bf16, two head-chains on disjoint PE row halves (concurrent K=64
   matmuls), 2 k-tiles per step -> one [128,2048] PSUM span, one exp call.
 - Mask applied multiplicatively in-place on VectorE (bf16 2x rate).
 - Ctx bf16 M=65 with a ones column producing the softmax denominator for
   free; both chains accumulate in one [65,1024] PSUM tile.
 - Normalization: reciprocal on VectorE, partition-broadcast on GpSimd,
   fused evac*recip on VectorE; chain B shifted to partitions 64:127 via
   SBUF->SBUF DMA (DVE is lane-locked).
 - Output projection (bf16, K=128) and pair-1 projections interleaved into
   the attention groups so the PE never idles; y returned as bf16 partials,
   host adds partials + bo + bv@Wo (exact: softmax rows sum to 1).
"""

import numpy as np
import ml_dtypes

import concourse.bacc as bacc
import concourse.mybir as mybir
import concourse.tile as tile
from concourse import bass_utils

BF = ml_dtypes.bfloat16
F8 = ml_dtypes.float8_e4m3fn
dt = mybir.dt
Exp = mybir.ActivationFunctionType.Exp
Copy = mybir.ActivationFunctionType.Copy
DR = mybir.MatmulPerfMode.DoubleRow
MUL = mybir.AluOpType.mult
ADD = mybir.AluOpType.add

NCORES = 8
W8SCALE = 64.0       # fp8 weight pre-scale (power of two)


def _emit(nc, tc, inp, y_d, S, E, HL, DK, dbg=None):
    NP = HL // 2              # head pairs (2)
    NT = S // 128             # seq tiles (16)
    EC = E // 128             # contraction chunks (8)
    NQ = S // 512             # q blocks (4)
    QB = 512                  # q block width

    persist = tc.alloc_tile_pool(name="persist", bufs=1)
    qT = [persist.tile([128, S], dt.bfloat16, name=f"qT{p}") for p in range(NP)]
    kT = [persist.tile([128, S], dt.bfloat16, name=f"kT{p}") for p in range(NP)]
    cT = [persist.tile([128, S], dt.bfloat16, name=f"cT{p}") for p in range(NP)]
    # v in natural layout: [seq-part, (t, head, 65)]; col 64 of each head
    # block preset to 1.0 (denominator ones column)
    vA = persist.tile([128, NT, 2 * NP, 65], dt.bfloat16, name="vA")
    nc.gpsimd.memset(vA[:], 1.0)
    neg3 = persist.tile([128, 1], dt.float32, name="neg3")
    nc.gpsimd.memset(neg3[:], -3.0)

    w_sb = {}
    for nm in ("wq", "wk", "wv"):
        w_sb[nm] = [persist.tile([128, 2 * NP * 64], dt.bfloat16, name=f"{nm}{c}")
                    for c in range(EC)]
    b_sb = {}
    for nm in ("bq", "bk"):
        b_sb[nm] = persist.tile([128, NP, 1], dt.float32, name=f"{nm}s")
        for p in range(NP):
            nc.sync.dma_start(b_sb[nm][:, p, :], inp[nm][p])
    wo_sb = [persist.tile([128, E], dt.bfloat16, name=f"wo{p}") for p in range(NP)]
    for p in range(NP):
        nc.sync.dma_start(wo_sb[p][:], inp["wo"][p])
    for nm in ("wk", "wq", "wv"):
        for c in range(EC):
            nc.sync.dma_start(w_sb[nm][c][:], inp[nm][c])

    xs = {}
    for nm in ("xq", "xk", "xv"):
        xs[nm] = [persist.tile([128, S], dt.bfloat16, name=f"{nm}{c}")
                  for c in range(EC)]

    mpool = tc.alloc_tile_pool(name="mask", bufs=4)
    aux = tc.alloc_tile_pool(name="aux", bufs=2, space="PSUM")
    espool = tc.alloc_tile_pool(name="es", bufs=5)
    erpool = tc.alloc_tile_pool(name="esr", bufs=2)
    stpool = tc.alloc_tile_pool(name="st", bufs=2, space="PSUM")
    ctxpool = tc.alloc_tile_pool(name="ctx", bufs=1, space="PSUM")
    npool = tc.alloc_tile_pool(name="nrm", bufs=1)
    ypool = tc.alloc_tile_pool(name="ysb", bufs=2)

    # ---- building-block emitters -------------------------------------------
    def qk_proj_unit(nm, pair, n0):
        """One n0-chunk of a q/k projection for one pair (4 DR MMs + evac)."""
        ps = aux.tile([128, 512], dt.float32, tag="aux", name=f"{nm}p{pair}_{n0}")
        w = w_sb["wq" if nm == "q" else "wk"]
        x = xs["xq" if nm == "q" else "xk"]
        for c in range(EC):
            nc.tensor.matmul(ps[:], w[c][:, 128 * pair:128 * (pair + 1)],
                             x[c][:, n0:n0 + 512],
                             start=(c == 0), stop=(c == EC - 1))
        dst = (qT if nm == "q" else kT)[pair][:, n0:n0 + 512]
        sc = 0.125 if nm == "q" else 1.0
        bias = b_sb["bq" if nm == "q" else "bk"][:, pair, :]
        nc.vector.tensor_scalar(dst, ps[:], sc, bias, MUL, ADD)

    def v_proj_unit(t):
        """v projection for one seq tile, all 4 heads (4 DR MMs + evac)."""
        ps = aux.tile([128, 512], dt.float32, tag="aux", name=f"vp{t}")
        for c in range(EC):
            nc.tensor.matmul(ps[:, 0:256],
                             xs["xv"][c][:, 128 * t:128 * (t + 1)],
                             w_sb["wv"][c][:],
                             start=(c == 0), stop=(c == EC - 1))
        nc.vector.tensor_copy(
            vA[:, t, :, 0:64],
            ps[:, 0:256].rearrange("p (h c) -> p h c", h=2 * NP))

    def outproj_unit(s, n0, on_act=False):
        """One (s-tile, E-chunk) of the output projection (2 MMs + evac + dma)."""
        ps = aux.tile([128, 512], dt.float32, tag="aux", name=f"y{s}_{n0}")
        for p in range(NP):
            nc.tensor.matmul(ps[:], cT[p][:, 128 * s:128 * (s + 1)],
                             wo_sb[p][:, n0:n0 + 512],
                             start=(p == 0), stop=(p == NP - 1))
        ysb = ypool.tile([128, 512], dt.bfloat16, tag="y", name=f"ysb{s}_{n0}")
        if on_act:
            nc.scalar.activation(ysb[:], ps[:], Copy)
        else:
            nc.vector.tensor_copy(ysb[:], ps[:])
        nc.sync.dma_start(y_d[128 * s:128 * (s + 1), n0:n0 + 512], ysb[:])

    # mask streamed as half-blocks [128, 8, 512] (4-deep ring, 32KB): the
    # ring refills mid-block so block boundaries see no mask-DMA hole.
    # The whole mask is re-read once per pair (does not fit SBUF at bf16).
    blocks = [(p, qb) for p in range(NP) for qb in range(NQ)]
    halves = [(p, qb, hh) for (p, qb) in blocks for hh in (0, 1)]
    mtiles = {}
    HNT = NT // 2

    def mask_half(idx):
        pair, qb, hh = halves[idx]
        mt = mpool.tile([128, HNT, 512], dt.bfloat16, tag="mask",
                        name=f"mt{pair}_{qb}_{hh}")
        for t in range(HNT):
            nc.sync.dma_start(mt[:, t, :],
                              inp["mask"][:, HNT * hh + t,
                                          512 * qb:512 * qb + 512])
        mtiles[idx] = mt

    # startup DMA interleave, ordered by first-consumer time: each entry is
    # a ~1MB unit (all 8 contraction chunks of one 512-col slice).
    def x_slice(nm, n0):
        for c in range(EC):
            nc.sync.dma_start(xs[nm][c][:, n0:n0 + 512],
                              inp[nm][c][:, n0:n0 + 512])

    x_slice("xk", 0)
    x_slice("xk", 512)
    x_slice("xq", 0)
    mask_half(0)
    x_slice("xq", 512)
    x_slice("xk", 1024)
    x_slice("xv", 0)
    mask_half(1)
    x_slice("xk", 1536)
    x_slice("xq", 1024)
    x_slice("xv", 512)
    x_slice("xq", 1536)
    x_slice("xv", 1024)
    x_slice("xv", 1536)
    mask_half(2)
    mask_req = [3]

    # ---- prologue: pair-0 q/k projections + first v tiles -------------------
    for n0 in range(0, S, 512):
        qk_proj_unit("k", 0, n0)
    for n0 in range(0, S, 512):
        qk_proj_unit("q", 0, n0)
    for t in range(4):
        v_proj_unit(t)

    # infill schedules per (pair, q-block): units emitted between attention
    # groups.  Ordering constraints: v(t) must precede ctx use (block 0 pops
    # 2/group, staying ahead of consumption); outproj for q-block b only
    # after pair-1 norm of block b (so it is scheduled during block b+1).
    infill = {
        (0, 0): [lambda t=t: v_proj_unit(t) for t in range(4, NT)],
        (0, 1): [lambda n0=n0: qk_proj_unit("k", 1, n0)
                 for n0 in range(0, S, 512)],
        (0, 2): [lambda n0=n0: qk_proj_unit("q", 1, n0)
                 for n0 in range(0, S, 512)],
        (1, 1): [lambda s=s, n0=n0: outproj_unit(s, n0)
                 for s in range(0, 4) for n0 in (0, 512)],
        (1, 2): [lambda s=s, n0=n0: outproj_unit(s, n0)
                 for s in range(4, 8) for n0 in (0, 512)],
        (1, 3): [lambda s=s, n0=n0: outproj_unit(s, n0)
                 for s in range(8, 12) for n0 in (0, 512)],
    }
    tail = [lambda s=s, n0=n0: outproj_unit(s, n0, on_act=True)
            for s in range(NT - 4, NT) for n0 in (0, 512)]

    # ---- main attention loop ------------------------------------------------
    for pair in range(NP):
        for qb in range(NQ):
            q0 = qb * QB
            units = infill.get((pair, qb), [])
            nu = len(units)
            bi = blocks.index((pair, qb))
            ctx2 = ctxpool.tile([65, 1024], dt.float32, tag="ctx",
                                name=f"ctx{pair}_{qb}")
            popped = 0
            for t in range(NT):
                # front-loaded infill: units for resource r must be emitted
                # strictly before their consumer (v(t) before ctx(t))
                target = -(-(nu * (t + 1)) // NT)
                while popped < target and units:
                    units.pop(0)()
                    popped += 1
                # keep the mask half-tile ring 3 ahead of the consumer
                hidx = 2 * bi + t // HNT
                want = hidx + 3
                while mask_req[0] <= want and mask_req[0] < len(halves):
                    mask_half(mask_req[0])
                    mask_req[0] += 1
                mt = mtiles[hidx]
                st = stpool.tile([128, 1024], dt.float32, tag="st",
                                 name=f"st{pair}_{qb}_{t}")
                # scores: chains on disjoint row halves issue concurrently
                for ch in range(2):
                    sub = 64 * ch
                    nc.tensor.matmul(
                        st[:, 512 * ch:512 * ch + 512],
                        kT[pair][sub:sub + 64, 128 * t:128 * (t + 1)],
                        qT[pair][sub:sub + 64, q0:q0 + QB],
                        start=True, stop=True)
                esr = erpool.tile([128, 1024], dt.bfloat16, tag="esr",
                                  name=f"esr{pair}_{qb}_{t}")
                nc.scalar.activation(esr[:], st[:], Exp, bias=neg3[:])
                es = espool.tile([128, 1024], dt.bfloat16, tag="es",
                                 name=f"es{pair}_{qb}_{t}")
                esv = es[:].rearrange("p (c n) -> p c n", c=2)
                nc.vector.tensor_mul(
                    esv, esr[:].rearrange("p (c n) -> p c n", c=2),
                    mt[:, t % HNT, :].unsqueeze(1).broadcast_to([128, 2, 512]))
                for ch in range(2):
                    h = 2 * pair + ch
                    nc.tensor.matmul(
                        ctx2[:, 512 * ch:512 * ch + 512],
                        vA[:, t, h, 0:65],
                        esv[:, ch], start=(t == 0), stop=(t == NT - 1))
                if t % HNT == HNT - 1:
                    mtiles.pop(hidx)
            # ---- normalization of this q block ------------------------------
            den = npool.tile([1, 1024], dt.float32, tag="dn", name=f"dn{pair}_{qb}")
            nc.vector.tensor_copy(den[:], ctx2[64:65, :])
            recip = npool.tile([1, 1024], dt.float32, tag="rc", name=f"rc{pair}_{qb}")
            nc.vector.reciprocal_approx_fast(recip[:], den[:])
            if dbg is not None and (pair, qb) == (0, 0):
                nc.sync.dma_start(dbg["den0"][:], den[:])
            bcast = npool.tile([64, 1024], dt.float32, tag="bc", name=f"bc{pair}_{qb}")
            nc.gpsimd.partition_broadcast(bcast[:], recip[:])
            nc.vector.tensor_mul(cT[pair][0:64, q0:q0 + QB],
                                 ctx2[0:64, 0:512], bcast[0:64, 0:512])
            tmpb = npool.tile([64, 512], dt.bfloat16, tag="tb", name=f"tb{pair}_{qb}")
            nc.vector.tensor_mul(tmpb[:], ctx2[0:64, 512:1024],
                                 bcast[0:64, 512:1024])
            nc.sync.dma_start(cT[pair][64:128, q0:q0 + QB], tmpb[:])

    for u in tail:
        u()
    if dbg is not None:
        for p in range(NP):
            nc.sync.dma_start(dbg[f"qT{p}"][:], qT[p][:])
            nc.sync.dma_start(dbg[f"kT{p}"][:], kT[p][:])
            nc.sync.dma_start(dbg[f"cT{p}"][:], cT[p][:])
        nc.sync.dma_start(dbg["vA"][:], vA[:].rearrange("p t h c -> p (t h c)"))

    ypool.release()
    npool.release()
    ctxpool.release()
    stpool.release()
    erpool.release()
    espool.release()
    aux.release()
    mpool.release()
    persist.release()


def _build(S, E, HL, DK):
    NP = HL // 2
    EC = E // 128
    NT = S // 128
    nc = bacc.Bacc("TRN2", target_bir_lowering=False, debug=False,
                   num_devices=NCORES)
    inp = {}
    for nm in ("xq", "xk", "xv"):
        inp[nm] = nc.dram_tensor(nm, [EC, 128, S], dt.bfloat16,
                                 kind="ExternalInput").ap()
    for nm in ("wq", "wk", "wv"):
        inp[nm] = nc.dram_tensor(nm, [EC, 128, 2 * NP * DK], dt.bfloat16,
                                 kind="ExternalInput").ap()
    for nm in ("bq", "bk"):
        inp[nm] = nc.dram_tensor(nm, [NP, 128, 1], dt.float32,
                                 kind="ExternalInput").ap()
    inp["wo"] = nc.dram_tensor("wo", [NP, 128, E], dt.bfloat16,
                               kind="ExternalInput").ap()
    inp["mask"] = nc.dram_tensor("mask", [128, NT, S], dt.bfloat16,
                                 kind="ExternalInput").ap()
    y_d = nc.dram_tensor("y", [S, E], dt.bfloat16, kind="ExternalOutput").ap()

    import os
    dbg = None
    if os.environ.get("K_DBG"):
        dbg = {}
        for p in range(NP):
            for nm in ("qT", "kT", "cT"):
                dbg[f"{nm}{p}"] = nc.dram_tensor(
                    f"dbg_{nm}{p}", [128, S], dt.bfloat16,
                    kind="ExternalOutput").ap()
        dbg["vA"] = nc.dram_tensor("dbg_vA", [128, NT * 2 * NP * 65],
                                   dt.bfloat16, kind="ExternalOutput").ap()
        dbg["den0"] = nc.dram_tensor("dbg_den0", [1, 1024], dt.float32,
                                     kind="ExternalOutput").ap()
    with tile.TileContext(nc) as tc:
        _emit(nc, tc, inp, y_d, S, E, HL, DK, dbg=dbg)
    nc.compile()
    return nc


_CACHE = {}
_TRACE = False
_TRACE_CORES = (0,)
_LAST_RESULT = None


def _get_nc(S, E, HL, DK):
    key = (S, E, HL, DK)
    if key not in _CACHE:
        _CACHE[key] = _build(S, E, HL, DK)
    return _CACHE[key]


_RUNNER_CACHE = {}


def _get_runner(nc):
    """Cached jitted shard_map executable (see bass2jax.run_bass_via_pjrt)."""
    if id(nc) in _RUNNER_CACHE:
        return _RUNNER_CACHE[id(nc)]
    import jax
    import concourse.mybir as _mybir
    from concourse import bass2jax
    from jax.sharding import Mesh, PartitionSpec
    from jax.experimental.shard_map import shard_map

    bass2jax.install_neuronx_cc_hook()
    pid_name = nc.partition_id_tensor.name if nc.partition_id_tensor else None
    in_names, out_names, out_avals, zero_shapes = [], [], [], []
    for alloc in nc.m.functions[0].allocations:
        if not isinstance(alloc, _mybir.MemoryLocationSet):
            continue
        name = alloc.memorylocations[0].name
        if alloc.kind == "ExternalInput":
            if name != pid_name:
                in_names.append(name)
        elif alloc.kind == "ExternalOutput":
            out_names.append(name)
            shape = tuple(alloc.tensor_shape)
            dtype = _mybir.dt.np(alloc.dtype)
            out_avals.append(jax.core.ShapedArray(shape, dtype))
            zero_shapes.append((shape, dtype))
    n_params = len(in_names)
    n_outs = len(out_avals)
    all_names = in_names + out_names
    if pid_name is not None:
        all_names = all_names + [pid_name]

    def _body(*args):
        operands = list(args)
        if pid_name is not None:
            operands.append(bass2jax.partition_id_tensor())
        return tuple(bass2jax._bass_exec_p.bind(
            *operands,
            out_avals=tuple(out_avals),
            in_names=tuple(all_names),
            out_names=tuple(out_names),
            lowering_input_output_aliases=(),
            sim_require_finite=True,
            sim_require_nnan=True,
            nc=nc,
        ))

    devices = jax.devices()[:NCORES]
    mesh = Mesh(np.asarray(devices), ("core",))
    donate = tuple(range(n_params, n_params + n_outs))
    sharded = jax.jit(
        shard_map(_body, mesh=mesh,
                  in_specs=(PartitionSpec("core"),) * (n_params + n_outs),
                  out_specs=(PartitionSpec("core"),) * n_outs,
                  check_rep=False),
        donate_argnums=donate, keep_unused=True)

    def run(in_maps):
        concat_in = [np.concatenate([np.asarray(m[nm]) for m in in_maps], axis=0)
                     for nm in in_names]
        concat_zeros = [np.zeros((NCORES * s[0], *s[1:]), d)
                        for s, d in zero_shapes]
        outs = sharded(*concat_in, *concat_zeros)
        return [
            {nm: np.asarray(outs[i]).reshape(NCORES, *out_avals[i].shape)[c]
             for i, nm in enumerate(out_names)}
            for c in range(NCORES)
        ]

    _RUNNER_CACHE[id(nc)] = run
    return run


def run_sharded(query, key, value, mask, Wq, bq, Wk, bk, Wv, bv, Wo, bo):
    global _LAST_RESULT
    query, key, value = (np.asarray(a, np.float32) for a in (query, key, value))
    mask = np.asarray(mask)
    Wq, bq, Wk, bk, Wv, bv, Wo, bo = (
        np.asarray(a, np.float32) for a in (Wq, bq, Wk, bk, Wv, bv, Wo, bo))

    B, S, E = query.shape
    HDK = Wq.shape[1]
    DKv = 64
    H = HDK // DKv
    GPB = NCORES // B                 # cores per batch (4)
    HL = H // GPB                     # heads per core (4)
    DKL = HL * DKv                    # local head dims (256)
    NP = HL // 2
    CP = E // 256
    NT = S // 128

    nc = _get_nc(S, E, HL, DKv)

    EC = E // 128
    xb = {}
    for b in range(B):
        xb[b] = {
            "xq": np.ascontiguousarray(query[b].T).astype(BF).reshape(EC, 128, S),
            "xk": np.ascontiguousarray(key[b].T).astype(BF).reshape(EC, 128, S),
            "xv": np.ascontiguousarray(value[b].T).astype(BF).reshape(EC, 128, S),
            "mask": np.ascontiguousarray(
                mask[b].T.reshape(NT, 128, S).transpose(1, 0, 2)).astype(BF),
        }

    in_maps = []
    for c in range(NCORES):
        b, g = c // GPB, c % GPB
        sl = slice(g * DKL, (g + 1) * DKL)
        in_maps.append({
            **xb[b],
            "wq": np.ascontiguousarray(Wq[:, sl]).astype(BF).reshape(EC, 128, DKL),
            "wk": np.ascontiguousarray(Wk[:, sl]).astype(BF).reshape(EC, 128, DKL),
            "wv": np.ascontiguousarray(Wv[:, sl]).astype(BF).reshape(EC, 128, DKL),
            "bq": (bq[sl] * 0.125).astype(np.float32).reshape(NP, 128, 1),
            "bk": bk[sl].astype(np.float32).reshape(NP, 128, 1),
            "wo": np.ascontiguousarray(Wo[sl, :]).astype(BF).reshape(NP, 128, E),
        })

    if _TRACE:
        res = bass_utils.run_bass_kernel_spmd(
            nc, in_maps, core_ids=list(range(NCORES)),
            trace=True, trace_cores=list(_TRACE_CORES))
        _LAST_RESULT = res
        results = res.results
    else:
        results = _get_runner(nc)(in_maps)

    y = np.zeros((B, S, E), np.float32)
    for c in range(NCORES):
        y[c // GPB] += np.asarray(results[c]["y"], np.float32)
    y += bo.astype(np.float32) + bv.astype(np.float32) @ Wo
    return y


def kernel(**inputs):
    return run_sharded(
        inputs["query"], inputs["key"], inputs["value"], inputs["mask"],
        inputs["Wq"], inputs["bq"], inputs["Wk"], inputs["bk"],
        inputs["Wv"], inputs["bv"], inputs["Wo"], inputs["bo"])


# revision 26
# speedup vs baseline: 1.2978x; 1.0692x over previous
"""Multi-head attention (B=2,S=2048,E=1024,H=16,DK=DV=64) on 8 Trainium2 cores.

Sharding: core c handles batch c//4 and head-group c%4 (4 heads = 2 pairs).
Fully software-pipelined single-pass kernel, engine-balanced around the
ScalarE exp stream (the hard floor: S^2*HL exps per core):

 - Projections in fp8e4 with DoubleRow matmuls (2x PE rate; weights scaled
   x64 into fp8 range, descaled at PSUM evacuation on VectorE, which also
   adds the bias via tensor_scalar's per-partition operand).
 - Scores bf16, two head-chains on disjoint PE row halves (concurrent K=64
   matmuls), 2 k-tiles per step -> one [128,2048] PSUM span, one exp call.
 - Mask applied multiplicatively in-place on VectorE (bf16 2x rate).
 - Ctx bf16 M=65 with a ones column producing the softmax denominator for
   free; both chains accumulate in one [65,1024] PSUM tile.
 - Normalization: reciprocal on VectorE, partition-broadcast on GpSimd,
   fused evac*recip on VectorE; chain B shifted to partitions 64:127 via
   SBUF->SBUF DMA (DVE is lane-locked).
 - Output projection (bf16, K=128) and pair-1 projections interleaved into
   the attention groups so the PE never idles; y returned as bf16 partials,
   host adds partials + bo + bv@Wo (exact: softmax rows sum to 1).
"""

import numpy as np
import ml_dtypes

import concourse.bacc as bacc
import concourse.mybir as mybir
import concourse.tile as tile
from concourse import bass_utils

BF = ml_dtypes.bfloat16
F8 = ml_dtypes.float8_e4m3fn
dt = mybir.dt
Exp = mybir.ActivationFunctionType.Exp
Copy = mybir.ActivationFunctionType.Copy
DR = mybir.MatmulPerfMode.DoubleRow
MUL = mybir.AluOpType.mult
ADD = mybir.AluOpType.add

NCORES = 8
W8SCALE = 64.0       # fp8 weight pre-scale (power of two)


def _emit(nc, tc, inp, y_d, S, E, HL, DK, dbg=None):
    NP = HL // 2              # head pairs (2)
    NT = S // 128             # seq tiles (16)
    EC = E // 128             # contraction chunks (8)
    NQ = S // 512             # q blocks (4)
    QB = 512                  # q block width

    persist = tc.alloc_tile_pool(name="persist", bufs=1)
    qT = [persist.tile([128, S], dt.bfloat16, name=f"qT{p}") for p in range(NP)]
    kT = [persist.tile([128, S], dt.bfloat16, name=f"kT{p}") for p in range(NP)]
    cT = [persist.tile([128, S], dt.bfloat16, name=f"cT{p}") for p in range(NP)]
    # v in natural layout: [seq-part, (t, head, 65)]; col 64 of each head
    # block preset to 1.0 (denominator ones column)
    vA = persist.tile([128, NT, 2 * NP, 65], dt.bfloat16, name="vA")
    nc.gpsimd.memset(vA[:], 1.0)
    neg3 = persist.tile([128, 1], dt.float32, name="neg3")
    nc.gpsimd.memset(neg3[:], -3.0)

    w_sb = {}
    for nm in ("wq", "wk", "wv"):
        w_sb[nm] = [persist.tile([128, 2 * NP * 64], dt.bfloat16, name=f"{nm}{c}")
                    for c in range(EC)]
    b_sb = {}
    for nm in ("bq", "bk"):
        b_sb[nm] = persist.tile([128, NP, 1], dt.float32, name=f"{nm}s")
        for p in range(NP):
            nc.sync.dma_start(b_sb[nm][:, p, :], inp[nm][p])
    wo_sb = [persist.tile([128, E], dt.bfloat16, name=f"wo{p}") for p in range(NP)]
    for p in range(NP):
        nc.sync.dma_start(wo_sb[p][:], inp["wo"][p])
    for nm in ("wk", "wq", "wv"):
        for c in range(EC):
            nc.sync.dma_start(w_sb[nm][c][:], inp[nm][c])

    xs = {}
    for nm in ("xq", "xk", "xv"):
        xs[nm] = [persist.tile([128, S], dt.bfloat16, name=f"{nm}{c}")
                  for c in range(EC)]

    mpool = tc.alloc_tile_pool(name="mask", bufs=4)
    aux = tc.alloc_tile_pool(name="aux", bufs=2, space="PSUM")
    espool = tc.alloc_tile_pool(name="es", bufs=5)
    erpool = tc.alloc_tile_pool(name="esr", bufs=2)
    stpool = tc.alloc_tile_pool(name="st", bufs=2, space="PSUM")
    ctxpool = tc.alloc_tile_pool(name="ctx", bufs=1, space="PSUM")
    npool = tc.alloc_tile_pool(name="nrm", bufs=1)
    ypool = tc.alloc_tile_pool(name="ysb", bufs=2)

    # ---- building-block emitters -------------------------------------------
    def qk_proj_unit(nm, pair, n0):
        """One n0-chunk of a q/k projection for one pair (4 DR MMs + evac)."""
        ps = aux.tile([128, 512], dt.float32, tag="aux", name=f"{nm}p{pair}_{n0}")
        w = w_sb["wq" if nm == "q" else "wk"]
        x = xs["xq" if nm == "q" else "xk"]
        for c in range(EC):
            nc.tensor.matmul(ps[:], w[c][:, 128 * pair:128 * (pair + 1)],
                             x[c][:, n0:n0 + 512],
                             start=(c == 0), stop=(c == EC - 1))
        dst = (qT if nm == "q" else kT)[pair][:, n0:n0 + 512]
        sc = 0.125 if nm == "q" else 1.0
        bias = b_sb["bq" if nm == "q" else "bk"][:, pair, :]
        nc.vector.tensor_scalar(dst, ps[:], sc, bias, MUL, ADD)

    def v_proj_unit(t):
        """v projection for one seq tile, all 4 heads (4 DR MMs + evac)."""
        ps = aux.tile([128, 512], dt.float32, tag="aux", name=f"vp{t}")
        for c in range(EC):
            nc.tensor.matmul(ps[:, 0:256],
                             xs["xv"][c][:, 128 * t:128 * (t + 1)],
                             w_sb["wv"][c][:],
                             start=(c == 0), stop=(c == EC - 1))
        nc.vector.tensor_copy(
            vA[:, t, :, 0:64],
            ps[:, 0:256].rearrange("p (h c) -> p h c", h=2 * NP))

    def outproj_unit(s, n0, on_act=False):
        """One (s-tile, E-chunk) of the output projection (2 MMs + evac + dma)."""
        ps = aux.tile([128, 512], dt.float32, tag="aux", name=f"y{s}_{n0}")
        for p in range(NP):
            nc.tensor.matmul(ps[:], cT[p][:, 128 * s:128 * (s + 1)],
                             wo_sb[p][:, n0:n0 + 512],
                             start=(p == 0), stop=(p == NP - 1))
        ysb = ypool.tile([128, 512], dt.bfloat16, tag="y", name=f"ysb{s}_{n0}")
        if on_act:
            nc.scalar.activation(ysb[:], ps[:], Copy)
        else:
            nc.vector.tensor_copy(ysb[:], ps[:])
        nc.sync.dma_start(y_d[128 * s:128 * (s + 1), n0:n0 + 512], ysb[:])

    # mask streamed as half-blocks [128, 8, 512] (4-deep ring, 32KB): the
    # ring refills mid-block so block boundaries see no mask-DMA hole.
    # The whole mask is re-read once per pair (does not fit SBUF at bf16).
    blocks = [(p, qb) for p in range(NP) for qb in range(NQ)]
    halves = [(p, qb, hh) for (p, qb) in blocks for hh in (0, 1)]
    mtiles = {}
    HNT = NT // 2

    def mask_half(idx):
        pair, qb, hh = halves[idx]
        mt = mpool.tile([128, HNT, 512], dt.bfloat16, tag="mask",
                        name=f"mt{pair}_{qb}_{hh}")
        nc.sync.dma_start(mt[:],
                          inp["mask"][:, HNT * hh:HNT * (hh + 1),
                                      512 * qb:512 * qb + 512])
        mtiles[idx] = mt

    # startup DMA: whole-chunk descriptors (DMA issue rate, not bandwidth,
    # limits the ramp), ordered by first consumer: xk -> xq -> mask -> xv.
    for c in range(EC):
        nc.sync.dma_start(xs["xk"][c][:], inp["xk"][c])
    for c in range(EC):
        nc.sync.dma_start(xs["xq"][c][:], inp["xq"][c])
    mask_half(0)
    mask_half(1)
    for c in range(EC):
        nc.sync.dma_start(xs["xv"][c][:], inp["xv"][c])
    mask_half(2)
    mask_req = [3]

    # ---- prologue: pair-0 q/k projections + first v tiles -------------------
    for n0 in range(0, S, 512):
        qk_proj_unit("k", 0, n0)
    for n0 in range(0, S, 512):
        qk_proj_unit("q", 0, n0)
    for t in range(4):
        v_proj_unit(t)

    # infill schedules per (pair, q-block): units emitted between attention
    # groups.  Ordering constraints: v(t) must precede ctx use (block 0 pops
    # 2/group, staying ahead of consumption); outproj for q-block b only
    # after pair-1 norm of block b (so it is scheduled during block b+1).
    infill = {
        (0, 0): [lambda t=t: v_proj_unit(t) for t in range(4, NT)],
        (0, 1): [lambda n0=n0: qk_proj_unit("k", 1, n0)
                 for n0 in range(0, S, 512)],
        (0, 2): [lambda n0=n0: qk_proj_unit("q", 1, n0)
                 for n0 in range(0, S, 512)],
        (1, 1): [lambda s=s, n0=n0: outproj_unit(s, n0)
                 for s in range(0, 4) for n0 in (0, 512)],
        (1, 2): [lambda s=s, n0=n0: outproj_unit(s, n0)
                 for s in range(4, 8) for n0 in (0, 512)],
        (1, 3): [lambda s=s, n0=n0: outproj_unit(s, n0)
                 for s in range(8, 12) for n0 in (0, 512)],
    }
    tail = [lambda s=s, n0=n0: outproj_unit(s, n0, on_act=True)
            for s in range(NT - 4, NT) for n0 in (0, 512)]

    # ---- main attention loop ------------------------------------------------
    for pair in range(NP):
        for qb in range(NQ):
            q0 = qb * QB
            units = infill.get((pair, qb), [])
            nu = len(units)
            bi = blocks.index((pair, qb))
            ctx2 = ctxpool.tile([65, 1024], dt.float32, tag="ctx",
                                name=f"ctx{pair}_{qb}")
            popped = 0
            for t in range(NT):
                # front-loaded infill: units for resource r must be emitted
                # strictly before their consumer (v(t) before ctx(t))
                target = -(-(nu * (t + 1)) // NT)
                while popped < target and units:
                    units.pop(0)()
                    popped += 1
                # keep the mask half-tile ring 3 ahead of the consumer
                hidx = 2 * bi + t // HNT
                want = hidx + 3
                while mask_req[0] <= want and mask_req[0] < len(halves):
                    mask_half(mask_req[0])
                    mask_req[0] += 1
                mt = mtiles[hidx]
                st = stpool.tile([128, 1024], dt.float32, tag="st",
                                 name=f"st{pair}_{qb}_{t}")
                # scores: chains on disjoint row halves issue concurrently
                for ch in range(2):
                    sub = 64 * ch
                    nc.tensor.matmul(
                        st[:, 512 * ch:512 * ch + 512],
                        kT[pair][sub:sub + 64, 128 * t:128 * (t + 1)],
                        qT[pair][sub:sub + 64, q0:q0 + QB],
                        start=True, stop=True)
                esr = erpool.tile([128, 1024], dt.bfloat16, tag="esr",
                                  name=f"esr{pair}_{qb}_{t}")
                nc.scalar.activation(esr[:], st[:], Exp, bias=neg3[:])
                es = espool.tile([128, 1024], dt.bfloat16, tag="es",
                                 name=f"es{pair}_{qb}_{t}")
                esv = es[:].rearrange("p (c n) -> p c n", c=2)
                nc.vector.tensor_mul(
                    esv, esr[:].rearrange("p (c n) -> p c n", c=2),
                    mt[:, t % HNT, :].unsqueeze(1).broadcast_to([128, 2, 512]))
                for ch in range(2):
                    h = 2 * pair + ch
                    nc.tensor.matmul(
                        ctx2[:, 512 * ch:512 * ch + 512],
                        vA[:, t, h, 0:65],
                        esv[:, ch], start=(t == 0), stop=(t == NT - 1))
                if t % HNT == HNT - 1:
                    mtiles.pop(hidx)
            # ---- normalization of this q block ------------------------------
            den = npool.tile([1, 1024], dt.float32, tag="dn", name=f"dn{pair}_{qb}")
            nc.vector.tensor_copy(den[:], ctx2[64:65, :])
            recip = npool.tile([1, 1024], dt.float32, tag="rc", name=f"rc{pair}_{qb}")
            nc.vector.reciprocal_approx_fast(recip[:], den[:])
            if dbg is not None and (pair, qb) == (0, 0):
                nc.sync.dma_start(dbg["den0"][:], den[:])
            bcast = npool.tile([64, 1024], dt.float32, tag="bc", name=f"bc{pair}_{qb}")
            nc.gpsimd.partition_broadcast(bcast[:], recip[:])
            nc.vector.tensor_mul(cT[pair][0:64, q0:q0 + QB],
                                 ctx2[0:64, 0:512], bcast[0:64, 0:512])
            tmpb = npool.tile([64, 512], dt.bfloat16, tag="tb", name=f"tb{pair}_{qb}")
            nc.vector.tensor_mul(tmpb[:], ctx2[0:64, 512:1024],
                                 bcast[0:64, 512:1024])
            nc.sync.dma_start(cT[pair][64:128, q0:q0 + QB], tmpb[:])

    for u in tail:
        u()
    if dbg is not None:
        for p in range(NP):
            nc.sync.dma_start(dbg[f"qT{p}"][:], qT[p][:])
            nc.sync.dma_start(dbg[f"kT{p}"][:], kT[p][:])
            nc.sync.dma_start(dbg[f"cT{p}"][:], cT[p][:])
        nc.sync.dma_start(dbg["vA"][:], vA[:].rearrange("p t h c -> p (t h c)"))

    ypool.release()
    npool.release()
    ctxpool.release()
    stpool.release()
    erpool.release()
    espool.release()
    aux.release()
    mpool.release()
    persist.release()


def _build(S, E, HL, DK):
    NP = HL // 2
    EC = E // 128
    NT = S // 128
    nc = bacc.Bacc("TRN2", target_bir_lowering=False, debug=False,
                   num_devices=NCORES)
    inp = {}
    for nm in ("xq", "xk", "xv"):
        inp[nm] = nc.dram_tensor(nm, [EC, 128, S], dt.bfloat16,
                                 kind="ExternalInput").ap()
    for nm in ("wq", "wk", "wv"):
        inp[nm] = nc.dram_tensor(nm, [EC, 128, 2 * NP * DK], dt.bfloat16,
                                 kind="ExternalInput").ap()
    for nm in ("bq", "bk"):
        inp[nm] = nc.dram_tensor(nm, [NP, 128, 1], dt.float32,
                                 kind="ExternalInput").ap()
    inp["wo"] = nc.dram_tensor("wo", [NP, 128, E], dt.bfloat16,
                               kind="ExternalInput").ap()
    inp["mask"] = nc.dram_tensor("mask", [128, NT, S], dt.bfloat16,
                                 kind="ExternalInput").ap()
    y_d = nc.dram_tensor("y", [S, E], dt.bfloat16, kind="ExternalOutput").ap()

    import os
    dbg = None
    if os.environ.get("K_DBG"):
        dbg = {}
        for p in range(NP):
            for nm in ("qT", "kT", "cT"):
                dbg[f"{nm}{p}"] = nc.dram_tensor(
                    f"dbg_{nm}{p}", [128, S], dt.bfloat16,
                    kind="ExternalOutput").ap()
        dbg["vA"] = nc.dram_tensor("dbg_vA", [128, NT * 2 * NP * 65],
                                   dt.bfloat16, kind="ExternalOutput").ap()
        dbg["den0"] = nc.dram_tensor("dbg_den0", [1, 1024], dt.float32,
                                     kind="ExternalOutput").ap()
    with tile.TileContext(nc) as tc:
        _emit(nc, tc, inp, y_d, S, E, HL, DK, dbg=dbg)
    nc.compile()
    return nc


_CACHE = {}
_TRACE = False
_TRACE_CORES = (0,)
_LAST_RESULT = None


def _get_nc(S, E, HL, DK):
    key = (S, E, HL, DK)
    if key not in _CACHE:
        _CACHE[key] = _build(S, E, HL, DK)
    return _CACHE[key]


_RUNNER_CACHE = {}


def _get_runner(nc):
    """Cached jitted shard_map executable (see bass2jax.run_bass_via_pjrt)."""
    if id(nc) in _RUNNER_CACHE:
        return _RUNNER_CACHE[id(nc)]
    import jax
    import concourse.mybir as _mybir
    from concourse import bass2jax
    from jax.sharding import Mesh, PartitionSpec
    from jax.experimental.shard_map import shard_map

    bass2jax.install_neuronx_cc_hook()
    pid_name = nc.partition_id_tensor.name if nc.partition_id_tensor else None
    in_names, out_names, out_avals, zero_shapes = [], [], [], []
    for alloc in nc.m.functions[0].allocations:
        if not isinstance(alloc, _mybir.MemoryLocationSet):
            continue
        name = alloc.memorylocations[0].name
        if alloc.kind == "ExternalInput":
            if name != pid_name:
                in_names.append(name)
        elif alloc.kind == "ExternalOutput":
            out_names.append(name)
            shape = tuple(alloc.tensor_shape)
            dtype = _mybir.dt.np(alloc.dtype)
            out_avals.append(jax.core.ShapedArray(shape, dtype))
            zero_shapes.append((shape, dtype))
    n_params = len(in_names)
    n_outs = len(out_avals)
    all_names = in_names + out_names
    if pid_name is not None:
        all_names = all_names + [pid_name]

    def _body(*args):
        operands = list(args)
        if pid_name is not None:
            operands.append(bass2jax.partition_id_tensor())
        return tuple(bass2jax._bass_exec_p.bind(
            *operands,
            out_avals=tuple(out_avals),
            in_names=tuple(all_names),
            out_names=tuple(out_names),
            lowering_input_output_aliases=(),
            sim_require_finite=True,
            sim_require_nnan=True,
            nc=nc,
        ))

    devices = jax.devices()[:NCORES]
    mesh = Mesh(np.asarray(devices), ("core",))
    donate = tuple(range(n_params, n_params + n_outs))
    sharded = jax.jit(
        shard_map(_body, mesh=mesh,
                  in_specs=(PartitionSpec("core"),) * (n_params + n_outs),
                  out_specs=(PartitionSpec("core"),) * n_outs,
                  check_rep=False),
        donate_argnums=donate, keep_unused=True)

    def run(in_maps):
        concat_in = [np.concatenate([np.asarray(m[nm]) for m in in_maps], axis=0)
                     for nm in in_names]
        concat_zeros = [np.zeros((NCORES * s[0], *s[1:]), d)
                        for s, d in zero_shapes]
        outs = sharded(*concat_in, *concat_zeros)
        return [
            {nm: np.asarray(outs[i]).reshape(NCORES, *out_avals[i].shape)[c]
             for i, nm in enumerate(out_names)}
            for c in range(NCORES)
        ]

    _RUNNER_CACHE[id(nc)] = run
    return run


def run_sharded(query, key, value, mask, Wq, bq, Wk, bk, Wv, bv, Wo, bo):
    global _LAST_RESULT
    query, key, value = (np.asarray(a, np.float32) for a in (query, key, value))
    mask = np.asarray(mask)
    Wq, bq, Wk, bk, Wv, bv, Wo, bo = (
        np.asarray(a, np.float32) for a in (Wq, bq, Wk, bk, Wv, bv, Wo, bo))

    B, S, E = query.shape
    HDK = Wq.shape[1]
    DKv = 64
    H = HDK // DKv
    GPB = NCORES // B                 # cores per batch (4)
    HL = H // GPB                     # heads per core (4)
    DKL = HL * DKv                    # local head dims (256)
    NP = HL // 2
    CP = E // 256
    NT = S // 128

    nc = _get_nc(S, E, HL, DKv)

    EC = E // 128
    xb = {}
    for b in range(B):
        xb[b] = {
            "xq": np.ascontiguousarray(query[b].T).astype(BF).reshape(EC, 128, S),
            "xk": np.ascontiguousarray(key[b].T).astype(BF).reshape(EC, 128, S),
            "xv": np.ascontiguousarray(value[b].T).astype(BF).reshape(EC, 128, S),
            "mask": np.ascontiguousarray(
                mask[b].T.reshape(NT, 128, S).transpose(1, 0, 2)).astype(BF),
        }

    in_maps = []
    for c in range(NCORES):
        b, g = c // GPB, c % GPB
        sl = slice(g * DKL, (g + 1) * DKL)
        in_maps.append({
            **xb[b],
            "wq": np.ascontiguousarray(Wq[:, sl]).astype(BF).reshape(EC, 128, DKL),
            "wk": np.ascontiguousarray(Wk[:, sl]).astype(BF).reshape(EC, 128, DKL),
            "wv": np.ascontiguousarray(Wv[:, sl]).astype(BF).reshape(EC, 128, DKL),
            "bq": (bq[sl] * 0.125).astype(np.float32).reshape(NP, 128, 1),
            "bk": bk[sl].astype(np.float32).reshape(NP, 128, 1),
            "wo": np.ascontiguousarray(Wo[sl, :]).astype(BF).reshape(NP, 128, E),
        })

    if _TRACE:
        res = bass_utils.run_bass_kernel_spmd(
            nc, in_maps, core_ids=list(range(NCORES)),
            trace=True, trace_cores=list(_TRACE_CORES))
        _LAST_RESULT = res
        results = res.results
    else:
        results = _get_runner(nc)(in_maps)

    y = np.zeros((B, S, E), np.float32)
    for c in range(NCORES):
        y[c // GPB] += np.asarray(results[c]["y"], np.float32)
    y += bo.astype(np.float32) + bv.astype(np.float32) @ Wo
    return y


def kernel(**inputs):
    return run_sharded(
        inputs["query"], inputs["key"], inputs["value"], inputs["mask"],
        inputs["Wq"], inputs["bq"], inputs["Wk"], inputs["bk"],
        inputs["Wv"], inputs["bv"], inputs["Wo"], inputs["bo"])


# revision 28
# speedup vs baseline: 1.3130x; 1.0117x over previous
"""Multi-head attention (B=2,S=2048,E=1024,H=16,DK=DV=64) on 8 Trainium2 cores.

Sharding: core c handles batch c//4 and head-group c%4 (4 heads = 2 pairs).
Fully software-pipelined single-pass kernel (all matmuls bf16 with fp32
PSUM accumulation; fp8 was tested and rejected: with Gaussian data any
~5%-relative quantization survives the averaging chain to the output):

 - Per-k-tile attention steps: two head-chains' K=64 score matmuls issue
   concurrently on disjoint PE row halves into one [128,1024] PSUM tile
   (double-buffered -> exp(t) overlaps scores(t+1)); one exp per step.
 - Mask applied on VectorE as a 3-op multiply with a stride-0 broadcast
   AP over the two chains (mask streamed from HBM as [128,8,512] halves
   in a 4-deep ring, single descriptors; re-read once per pair).
 - Ctx bf16 M=65 with a ones column producing the softmax denominator for
   free; both chains accumulate in one [65,1024] PSUM tile.
 - Normalization: den copy + reciprocal_approx_fast on VectorE (the custom
   DVE op cannot read PSUM directly), partition-broadcast on GpSimd, fused
   evac*recip on VectorE; chain B shifted to partitions 64:127 via
   SBUF->SBUF DMA (DVE cannot re-base partitions on write).
 - q/k/v projections (bias folded into the PSUM evacuation via
   tensor_scalar's per-partition operand) and the output projection are
   interleaved into the attention steps as "infill units", front-loaded so
   producers always precede consumers; y returned as bf16 partials, host
   adds partials + bo + bv@Wo (exact: softmax rows sum to 1).
 - Startup DMAs use whole-chunk descriptors ordered by first consumer
   (descriptor issue rate, not bandwidth, limits the ramp).
PSUM budget: scores 2x2 banks + ctx 2 + proj/outproj aux 2 = 8.
"""

import numpy as np
import ml_dtypes

import concourse.bacc as bacc
import concourse.mybir as mybir
import concourse.tile as tile
from concourse import bass_utils

BF = ml_dtypes.bfloat16
F8 = ml_dtypes.float8_e4m3fn
dt = mybir.dt
Exp = mybir.ActivationFunctionType.Exp
Copy = mybir.ActivationFunctionType.Copy
DR = mybir.MatmulPerfMode.DoubleRow
MUL = mybir.AluOpType.mult
ADD = mybir.AluOpType.add

NCORES = 8
W8SCALE = 64.0       # fp8 weight pre-scale (power of two)


def _emit(nc, tc, inp, y_d, S, E, HL, DK, dbg=None):
    NP = HL // 2              # head pairs (2)
    NT = S // 128             # seq tiles (16)
    EC = E // 128             # contraction chunks (8)
    NQ = S // 512             # q blocks (4)
    QB = 512                  # q block width

    persist = tc.alloc_tile_pool(name="persist", bufs=1)
    qT = [persist.tile([128, S], dt.bfloat16, name=f"qT{p}") for p in range(NP)]
    kT = [persist.tile([128, S], dt.bfloat16, name=f"kT{p}") for p in range(NP)]
    cT = [persist.tile([128, S], dt.bfloat16, name=f"cT{p}") for p in range(NP)]
    # v in natural layout: [seq-part, (t, head, 65)]; col 64 of each head
    # block preset to 1.0 (denominator ones column)
    vA = persist.tile([128, NT, 2 * NP, 65], dt.bfloat16, name="vA")
    nc.gpsimd.memset(vA[:], 1.0)
    neg3 = persist.tile([128, 1], dt.float32, name="neg3")
    nc.gpsimd.memset(neg3[:], -3.0)

    w_sb = {}
    for nm in ("wq", "wk", "wv"):
        w_sb[nm] = [persist.tile([128, 2 * NP * 64], dt.bfloat16, name=f"{nm}{c}")
                    for c in range(EC)]
    b_sb = {}
    for nm in ("bq", "bk"):
        b_sb[nm] = persist.tile([128, NP, 1], dt.float32, name=f"{nm}s")
        for p in range(NP):
            nc.sync.dma_start(b_sb[nm][:, p, :], inp[nm][p])
    wo_sb = [persist.tile([128, E], dt.bfloat16, name=f"wo{p}") for p in range(NP)]
    for p in range(NP):
        nc.sync.dma_start(wo_sb[p][:], inp["wo"][p])
    for nm in ("wk", "wq", "wv"):
        for c in range(EC):
            nc.sync.dma_start(w_sb[nm][c][:], inp[nm][c])

    xs = {}
    for nm in ("xq", "xk", "xv"):
        xs[nm] = [persist.tile([128, S], dt.bfloat16, name=f"{nm}{c}")
                  for c in range(EC)]

    mpool = tc.alloc_tile_pool(name="mask", bufs=4)
    aux = tc.alloc_tile_pool(name="aux", bufs=2, space="PSUM")
    espool = tc.alloc_tile_pool(name="es", bufs=5)
    erpool = tc.alloc_tile_pool(name="esr", bufs=2)
    stpool = tc.alloc_tile_pool(name="st", bufs=2, space="PSUM")
    ctxpool = tc.alloc_tile_pool(name="ctx", bufs=1, space="PSUM")
    npool = tc.alloc_tile_pool(name="nrm", bufs=1)
    ypool = tc.alloc_tile_pool(name="ysb", bufs=2)

    # ---- building-block emitters -------------------------------------------
    def qk_proj_unit(nm, pair, n0):
        """One n0-chunk of a q/k projection for one pair (4 DR MMs + evac)."""
        ps = aux.tile([128, 512], dt.float32, tag="aux", name=f"{nm}p{pair}_{n0}")
        w = w_sb["wq" if nm == "q" else "wk"]
        x = xs["xq" if nm == "q" else "xk"]
        for c in range(EC):
            nc.tensor.matmul(ps[:], w[c][:, 128 * pair:128 * (pair + 1)],
                             x[c][:, n0:n0 + 512],
                             start=(c == 0), stop=(c == EC - 1))
        dst = (qT if nm == "q" else kT)[pair][:, n0:n0 + 512]
        sc = 0.125 if nm == "q" else 1.0
        bias = b_sb["bq" if nm == "q" else "bk"][:, pair, :]
        nc.vector.tensor_scalar(dst, ps[:], sc, bias, MUL, ADD)

    def v_proj_unit(t):
        """v projection for one seq tile, all 4 heads (4 DR MMs + evac)."""
        ps = aux.tile([128, 512], dt.float32, tag="aux", name=f"vp{t}")
        for c in range(EC):
            nc.tensor.matmul(ps[:, 0:256],
                             xs["xv"][c][:, 128 * t:128 * (t + 1)],
                             w_sb["wv"][c][:],
                             start=(c == 0), stop=(c == EC - 1))
        nc.vector.tensor_copy(
            vA[:, t, :, 0:64],
            ps[:, 0:256].rearrange("p (h c) -> p h c", h=2 * NP))

    def outproj_unit(s, n0, on_act=False):
        """One (s-tile, E-chunk) of the output projection (2 MMs + evac + dma)."""
        ps = aux.tile([128, 512], dt.float32, tag="aux", name=f"y{s}_{n0}")
        for p in range(NP):
            nc.tensor.matmul(ps[:], cT[p][:, 128 * s:128 * (s + 1)],
                             wo_sb[p][:, n0:n0 + 512],
                             start=(p == 0), stop=(p == NP - 1))
        ysb = ypool.tile([128, 512], dt.bfloat16, tag="y", name=f"ysb{s}_{n0}")
        if on_act:
            nc.scalar.activation(ysb[:], ps[:], Copy)
        else:
            nc.vector.tensor_copy(ysb[:], ps[:])
        nc.sync.dma_start(y_d[128 * s:128 * (s + 1), n0:n0 + 512], ysb[:])

    # mask streamed as half-blocks [128, 8, 512] (4-deep ring, 32KB): the
    # ring refills mid-block so block boundaries see no mask-DMA hole.
    # The whole mask is re-read once per pair (does not fit SBUF at bf16).
    blocks = [(p, qb) for p in range(NP) for qb in range(NQ)]
    halves = [(p, qb, hh) for (p, qb) in blocks for hh in (0, 1)]
    mtiles = {}
    HNT = NT // 2

    def mask_half(idx):
        pair, qb, hh = halves[idx]
        mt = mpool.tile([128, HNT, 512], dt.bfloat16, tag="mask",
                        name=f"mt{pair}_{qb}_{hh}")
        nc.sync.dma_start(mt[:],
                          inp["mask"][:, HNT * hh:HNT * (hh + 1),
                                      512 * qb:512 * qb + 512])
        mtiles[idx] = mt

    # startup DMA: whole-chunk descriptors (DMA issue rate, not bandwidth,
    # limits the ramp), ordered by first consumer: xk -> xq -> mask -> xv.
    for c in range(EC):
        nc.sync.dma_start(xs["xk"][c][:], inp["xk"][c])
    for c in range(EC):
        nc.sync.dma_start(xs["xq"][c][:], inp["xq"][c])
    mask_half(0)
    mask_half(1)
    for c in range(EC):
        nc.sync.dma_start(xs["xv"][c][:], inp["xv"][c])
    mask_half(2)
    mask_req = [3]

    # ---- prologue: pair-0 q/k projections + first v tiles -------------------
    for n0 in range(0, S, 512):
        qk_proj_unit("k", 0, n0)
    for n0 in range(0, S, 512):
        qk_proj_unit("q", 0, n0)
    for t in range(4):
        v_proj_unit(t)

    # infill schedules per (pair, q-block): units emitted between attention
    # groups.  Ordering constraints: v(t) must precede ctx use (block 0 pops
    # 2/group, staying ahead of consumption); outproj for q-block b only
    # after pair-1 norm of block b (so it is scheduled during block b+1).
    infill = {
        (0, 0): [lambda t=t: v_proj_unit(t) for t in range(4, NT)],
        (0, 1): [lambda n0=n0: qk_proj_unit("k", 1, n0)
                 for n0 in range(0, S, 512)],
        (0, 2): [lambda n0=n0: qk_proj_unit("q", 1, n0)
                 for n0 in range(0, S, 512)],
        (1, 1): [lambda s=s, n0=n0: outproj_unit(s, n0)
                 for s in range(0, 4) for n0 in (0, 512)],
        (1, 2): [lambda s=s, n0=n0: outproj_unit(s, n0)
                 for s in range(4, 8) for n0 in (0, 512)],
        (1, 3): [lambda s=s, n0=n0: outproj_unit(s, n0)
                 for s in range(8, 12) for n0 in (0, 512)],
    }
    tail = [lambda s=s, n0=n0: outproj_unit(s, n0, on_act=True)
            for s in range(NT - 4, NT) for n0 in (0, 512)]

    # ---- main attention loop ------------------------------------------------
    for pair in range(NP):
        for qb in range(NQ):
            q0 = qb * QB
            units = infill.get((pair, qb), [])
            nu = len(units)
            bi = blocks.index((pair, qb))
            ctx2 = ctxpool.tile([65, 1024], dt.float32, tag="ctx",
                                name=f"ctx{pair}_{qb}")
            popped = 0
            for t in range(NT):
                # front-loaded infill: units for resource r must be emitted
                # strictly before their consumer (v(t) before ctx(t))
                target = -(-(nu * (t + 1)) // NT)
                while popped < target and units:
                    # emitted early for dependency order, but scheduled as
                    # gap-filler behind the scores/exp/ctx critical loop
                    with tc.high_priority(offset=-40):
                        units.pop(0)()
                    popped += 1
                # keep the mask half-tile ring 3 ahead of the consumer
                hidx = 2 * bi + t // HNT
                want = hidx + 3
                while mask_req[0] <= want and mask_req[0] < len(halves):
                    mask_half(mask_req[0])
                    mask_req[0] += 1
                mt = mtiles[hidx]
                st = stpool.tile([128, 1024], dt.float32, tag="st",
                                 name=f"st{pair}_{qb}_{t}")
                # scores: chains on disjoint row halves issue concurrently
                for ch in range(2):
                    sub = 64 * ch
                    nc.tensor.matmul(
                        st[:, 512 * ch:512 * ch + 512],
                        kT[pair][sub:sub + 64, 128 * t:128 * (t + 1)],
                        qT[pair][sub:sub + 64, q0:q0 + QB],
                        start=True, stop=True)
                esr = erpool.tile([128, 1024], dt.bfloat16, tag="esr",
                                  name=f"esr{pair}_{qb}_{t}")
                nc.scalar.activation(esr[:], st[:], Exp, bias=neg3[:])
                es = espool.tile([128, 1024], dt.bfloat16, tag="es",
                                 name=f"es{pair}_{qb}_{t}")
                esv = es[:].rearrange("p (c n) -> p c n", c=2)
                nc.vector.tensor_mul(
                    esv, esr[:].rearrange("p (c n) -> p c n", c=2),
                    mt[:, t % HNT, :].unsqueeze(1).broadcast_to([128, 2, 512]))
                for ch in range(2):
                    h = 2 * pair + ch
                    nc.tensor.matmul(
                        ctx2[:, 512 * ch:512 * ch + 512],
                        vA[:, t, h, 0:65],
                        esv[:, ch], start=(t == 0), stop=(t == NT - 1))
                if t % HNT == HNT - 1:
                    mtiles.pop(hidx)
            # ---- normalization of this q block ------------------------------
            den = npool.tile([1, 1024], dt.float32, tag="dn", name=f"dn{pair}_{qb}")
            nc.vector.tensor_copy(den[:], ctx2[64:65, :])
            recip = npool.tile([1, 1024], dt.float32, tag="rc", name=f"rc{pair}_{qb}")
            nc.vector.reciprocal_approx_fast(recip[:], den[:])
            if dbg is not None and (pair, qb) == (0, 0):
                nc.sync.dma_start(dbg["den0"][:], den[:])
            bcast = npool.tile([64, 1024], dt.float32, tag="bc", name=f"bc{pair}_{qb}")
            nc.gpsimd.partition_broadcast(bcast[:], recip[:])
            nc.vector.tensor_mul(cT[pair][0:64, q0:q0 + QB],
                                 ctx2[0:64, 0:512], bcast[0:64, 0:512])
            tmpb = npool.tile([64, 512], dt.bfloat16, tag="tb", name=f"tb{pair}_{qb}")
            nc.vector.tensor_mul(tmpb[:], ctx2[0:64, 512:1024],
                                 bcast[0:64, 512:1024])
            nc.sync.dma_start(cT[pair][64:128, q0:q0 + QB], tmpb[:])

    for u in tail:
        u()
    if dbg is not None:
        for p in range(NP):
            nc.sync.dma_start(dbg[f"qT{p}"][:], qT[p][:])
            nc.sync.dma_start(dbg[f"kT{p}"][:], kT[p][:])
            nc.sync.dma_start(dbg[f"cT{p}"][:], cT[p][:])
        nc.sync.dma_start(dbg["vA"][:], vA[:].rearrange("p t h c -> p (t h c)"))

    ypool.release()
    npool.release()
    ctxpool.release()
    stpool.release()
    erpool.release()
    espool.release()
    aux.release()
    mpool.release()
    persist.release()


def _build(S, E, HL, DK):
    NP = HL // 2
    EC = E // 128
    NT = S // 128
    nc = bacc.Bacc("TRN2", target_bir_lowering=False, debug=False,
                   num_devices=NCORES)
    inp = {}
    for nm in ("xq", "xk", "xv"):
        inp[nm] = nc.dram_tensor(nm, [EC, 128, S], dt.bfloat16,
                                 kind="ExternalInput").ap()
    for nm in ("wq", "wk", "wv"):
        inp[nm] = nc.dram_tensor(nm, [EC, 128, 2 * NP * DK], dt.bfloat16,
                                 kind="ExternalInput").ap()
    for nm in ("bq", "bk"):
        inp[nm] = nc.dram_tensor(nm, [NP, 128, 1], dt.float32,
                                 kind="ExternalInput").ap()
    inp["wo"] = nc.dram_tensor("wo", [NP, 128, E], dt.bfloat16,
                               kind="ExternalInput").ap()
    inp["mask"] = nc.dram_tensor("mask", [128, NT, S], dt.bfloat16,
                                 kind="ExternalInput").ap()
    y_d = nc.dram_tensor("y", [S, E], dt.bfloat16, kind="ExternalOutput").ap()

    import os
    dbg = None
    if os.environ.get("K_DBG"):
        dbg = {}
        for p in range(NP):
            for nm in ("qT", "kT", "cT"):
                dbg[f"{nm}{p}"] = nc.dram_tensor(
                    f"dbg_{nm}{p}", [128, S], dt.bfloat16,
                    kind="ExternalOutput").ap()
        dbg["vA"] = nc.dram_tensor("dbg_vA", [128, NT * 2 * NP * 65],
                                   dt.bfloat16, kind="ExternalOutput").ap()
        dbg["den0"] = nc.dram_tensor("dbg_den0", [1, 1024], dt.float32,
                                     kind="ExternalOutput").ap()
    with tile.TileContext(nc) as tc:
        _emit(nc, tc, inp, y_d, S, E, HL, DK, dbg=dbg)
    nc.compile()
    return nc


_CACHE = {}
_TRACE = False
_TRACE_CORES = (0,)
_LAST_RESULT = None


def _get_nc(S, E, HL, DK):
    key = (S, E, HL, DK)
    if key not in _CACHE:
        _CACHE[key] = _build(S, E, HL, DK)
    return _CACHE[key]


_RUNNER_CACHE = {}


def _get_runner(nc):
    """Cached jitted shard_map executable (see bass2jax.run_bass_via_pjrt)."""
    if id(nc) in _RUNNER_CACHE:
        return _RUNNER_CACHE[id(nc)]
    import jax
    import concourse.mybir as _mybir
    from concourse import bass2jax
    from jax.sharding import Mesh, PartitionSpec
    from jax.experimental.shard_map import shard_map

    bass2jax.install_neuronx_cc_hook()
    pid_name = nc.partition_id_tensor.name if nc.partition_id_tensor else None
    in_names, out_names, out_avals, zero_shapes = [], [], [], []
    for alloc in nc.m.functions[0].allocations:
        if not isinstance(alloc, _mybir.MemoryLocationSet):
            continue
        name = alloc.memorylocations[0].name
        if alloc.kind == "ExternalInput":
            if name != pid_name:
                in_names.append(name)
        elif alloc.kind == "ExternalOutput":
            out_names.append(name)
            shape = tuple(alloc.tensor_shape)
            dtype = _mybir.dt.np(alloc.dtype)
            out_avals.append(jax.core.ShapedArray(shape, dtype))
            zero_shapes.append((shape, dtype))
    n_params = len(in_names)
    n_outs = len(out_avals)
    all_names = in_names + out_names
    if pid_name is not None:
        all_names = all_names + [pid_name]

    def _body(*args):
        operands = list(args)
        if pid_name is not None:
            operands.append(bass2jax.partition_id_tensor())
        return tuple(bass2jax._bass_exec_p.bind(
            *operands,
            out_avals=tuple(out_avals),
            in_names=tuple(all_names),
            out_names=tuple(out_names),
            lowering_input_output_aliases=(),
            sim_require_finite=True,
            sim_require_nnan=True,
            nc=nc,
        ))

    devices = jax.devices()[:NCORES]
    mesh = Mesh(np.asarray(devices), ("core",))
    donate = tuple(range(n_params, n_params + n_outs))
    sharded = jax.jit(
        shard_map(_body, mesh=mesh,
                  in_specs=(PartitionSpec("core"),) * (n_params + n_outs),
                  out_specs=(PartitionSpec("core"),) * n_outs,
                  check_rep=False),
        donate_argnums=donate, keep_unused=True)

    def run(in_maps):
        concat_in = [np.concatenate([np.asarray(m[nm]) for m in in_maps], axis=0)
                     for nm in in_names]
        concat_zeros = [np.zeros((NCORES * s[0], *s[1:]), d)
                        for s, d in zero_shapes]
        outs = sharded(*concat_in, *concat_zeros)
        return [
            {nm: np.asarray(outs[i]).reshape(NCORES, *out_avals[i].shape)[c]
             for i, nm in enumerate(out_names)}
            for c in range(NCORES)
        ]

    _RUNNER_CACHE[id(nc)] = run
    return run


def run_sharded(query, key, value, mask, Wq, bq, Wk, bk, Wv, bv, Wo, bo):
    global _LAST_RESULT
    query, key, value = (np.asarray(a, np.float32) for a in (query, key, value))
    mask = np.asarray(mask)
    Wq, bq, Wk, bk, Wv, bv, Wo, bo = (
        np.asarray(a, np.float32) for a in (Wq, bq, Wk, bk, Wv, bv, Wo, bo))

    B, S, E = query.shape
    HDK = Wq.shape[1]
    DKv = 64
    H = HDK // DKv
    GPB = NCORES // B                 # cores per batch (4)
    HL = H // GPB                     # heads per core (4)
    DKL = HL * DKv                    # local head dims (256)
    NP = HL // 2
    CP = E // 256
    NT = S // 128

    nc = _get_nc(S, E, HL, DKv)

    EC = E // 128
    xb = {}
    for b in range(B):
        xb[b] = {
            "xq": np.ascontiguousarray(query[b].T).astype(BF).reshape(EC, 128, S),
            "xk": np.ascontiguousarray(key[b].T).astype(BF).reshape(EC, 128, S),
            "xv": np.ascontiguousarray(value[b].T).astype(BF).reshape(EC, 128, S),
            "mask": np.ascontiguousarray(
                mask[b].T.reshape(NT, 128, S).transpose(1, 0, 2)).astype(BF),
        }

    in_maps = []
    for c in range(NCORES):
        b, g = c // GPB, c % GPB
        sl = slice(g * DKL, (g + 1) * DKL)
        in_maps.append({
            **xb[b],
            "wq": np.ascontiguousarray(Wq[:, sl]).astype(BF).reshape(EC, 128, DKL),
            "wk": np.ascontiguousarray(Wk[:, sl]).astype(BF).reshape(EC, 128, DKL),
            "wv": np.ascontiguousarray(Wv[:, sl]).astype(BF).reshape(EC, 128, DKL),
            "bq": (bq[sl] * 0.125).astype(np.float32).reshape(NP, 128, 1),
            "bk": bk[sl].astype(np.float32).reshape(NP, 128, 1),
            "wo": np.ascontiguousarray(Wo[sl, :]).astype(BF).reshape(NP, 128, E),
        })

    if _TRACE:
        res = bass_utils.run_bass_kernel_spmd(
            nc, in_maps, core_ids=list(range(NCORES)),
            trace=True, trace_cores=list(_TRACE_CORES))
        _LAST_RESULT = res
        results = res.results
    else:
        results = _get_runner(nc)(in_maps)

    y = np.zeros((B, S, E), np.float32)
    for c in range(NCORES):
        y[c // GPB] += np.asarray(results[c]["y"], np.float32)
    y += bo.astype(np.float32) + bv.astype(np.float32) @ Wo
    return y


def kernel(**inputs):
    return run_sharded(
        inputs["query"], inputs["key"], inputs["value"], inputs["mask"],
        inputs["Wq"], inputs["bq"], inputs["Wk"], inputs["bk"],
        inputs["Wv"], inputs["bv"], inputs["Wo"], inputs["bo"])
